# revision 9
# baseline (speedup 1.0000x reference)
"""Euclidean distance loss (mean over all pairs ||C[i]-D[j]||_F) on 8 TRN2 cores.

Strategy:
  mean_ij ||C_i - D_j|| with ||c-d||^2 = ||c||^2 + ||d||^2 - 2<c,d>.
  The gram term is one big GEMM: [1024 x 16384] @ [16384 x 1024].

  Augmented-GEMM trick: append 4 rows to the contraction dim carrying
  (c_sq - 16384) split hi/lo against ones, and ones against (d_sq - 16384)
  hi/lo, so PSUM accumulates  (c_sq-16384)+(d_sq-16384)-2<c,d>  directly.
  The epilogue is then sqrt(x + 32768) + row-sum in a single scalar-engine
  activation per PSUM tile.

  Sharding: 4 i-blocks (256 rows of C) x 2 j-blocks (512 rows of D) over the
  8 cores -> per-core HBM traffic 25.3 MB in bf16 (vs 38 MB for 1D row
  sharding).  Data is transposed on the host to [d, n] layout so both matmul
  operands land with the contraction dim on partitions via fully contiguous
  DMAs; the bf16 cast halves HBM bytes (error on the final mean ~1e-6 rel:
  norms are carried exactly via fp64->bf16 hi/lo augmentation rows, and the
  gram noise is zero-mean and averages out over the 2^20 pairs).

  Per core: 260 matmuls (130 K-chunks x 2 i-subblocks, N=512, bf16) into two
  PSUM banks, K-contiguous so the PE stays warm; DMA streams in 10 groups of
  13 K-chunks (0.83 MB + 1.66 MB per group) with bufs=4 prefetch.
"""

import sys
import numpy as np

for _p in ("/opt/trn_rl_repo", "/root/.axon_site/_ro/trn_rl_repo"):
    if _p not in sys.path:
        sys.path.insert(0, _p)

import ml_dtypes

BF16 = ml_dtypes.bfloat16

N = 1024            # rows of C and of D
DDIM = 128 * 128    # flattened feature dim = 16384
P = 128             # SBUF partitions / matmul contraction per chunk
CH = 13             # K-chunks per DMA group
G = 10              # DMA groups
DAUG = G * CH * P   # padded contraction dim = 16640 (16384 data + 4 aug + pad)
NI = 256            # i-columns per core (4 i-blocks)
NJ = 512            # j-columns per core (2 j-blocks)
NCORES = 8


def _build_nc(hw=True):
    import concourse.bass as bass
    import concourse.mybir as mybir
    import concourse.tile as tile

    nc = bass.Bass("TRN2")
    ct_d = nc.dram_tensor("ct", [G, P, CH, NI], mybir.dt.bfloat16, kind="ExternalInput")
    dt_d = nc.dram_tensor("dt", [G, P, CH, NJ], mybir.dt.bfloat16, kind="ExternalInput")
    out_d = nc.dram_tensor("out", [P, 2], mybir.dt.float32, kind="ExternalOutput")

    with tile.TileContext(nc) as tc:
        with (
            tc.tile_pool(name="ctp", bufs=4) as ct_pool,
            tc.tile_pool(name="dtp", bufs=4) as dt_pool,
            tc.tile_pool(name="eps", bufs=1) as eps_pool,
            tc.tile_pool(name="ps", bufs=1, space=bass.MemorySpace.PSUM) as psum_pool,
        ):
            ps0 = psum_pool.tile([P, NJ], mybir.dt.float32)
            ps1 = psum_pool.tile([P, NJ], mybir.dt.float32)
            nchunks = G * CH
            for g in range(G):
                ct_t = ct_pool.tile([P, CH, NI], mybir.dt.bfloat16)
                dt_t = dt_pool.tile([P, CH, NJ], mybir.dt.bfloat16)
                nc.gpsimd.dma_start(ct_t[:], ct_d[g])
                nc.gpsimd.dma_start(dt_t[:], dt_d[g])
                for n in range(CH):
                    k = g * CH + n
                    start = k == 0
                    stop = k == nchunks - 1
                    nc.tensor.matmul(
                        ps0[:], ct_t[:, n, 0:128], dt_t[:, n, :], start=start, stop=stop
                    )
                    nc.tensor.matmul(
                        ps1[:], ct_t[:, n, 128:256], dt_t[:, n, :], start=start, stop=stop
                    )
            acc = eps_pool.tile([P, 2], mybir.dt.float32)
            dist0 = eps_pool.tile([P, NJ], mybir.dt.float32)
            dist1 = eps_pool.tile([P, NJ], mybir.dt.float32)
            # bias=0.0 uses the preamble-initialized const AP (behind an
            # all-engine barrier), so the activations only wait on PE.
            nc.scalar.activation(
                dist0[:], ps0[:], mybir.ActivationFunctionType.Sqrt,
                bias=0.0, accum_out=acc[:, 0:1],
            )
            nc.scalar.activation(
                dist1[:], ps1[:], mybir.ActivationFunctionType.Sqrt,
                bias=0.0, accum_out=acc[:, 1:2],
            )
            nc.sync.dma_start(out_d[:], acc[:])

    if hw:
        # Post-passes that satisfy walrus' per-instruction sync-wait caps.
        # They only touch scheduling metadata (sem waits), not dataflow, but
        # the CoreSim race detector can't digest hand-inserted instructions,
        # so sim_test builds with hw=False.
        _strip_redundant_dma_waits(nc)
        _split_multiwait_drains(nc, mybir)
    return nc


def _split_multiwait_drains(nc, mybir):
    """walrus CTRL structs also cap sync-wait commands per instruction; the
    Tile kernel-tail drain waits on every sem used (11 here).  Hoist all but
    the last wait onto single-wait NoOps queued immediately before the drain
    on the same engine — sequencer program order makes this equivalent."""
    for blk in nc.m.functions[0].blocks:
        insts = blk.instructions
        i = 0
        while i < len(insts):
            ins = insts[i]
            si = getattr(ins, "sync_info", None)
            if (
                type(ins).__name__ == "InstDrain"
                and si is not None
                and len(si.on_wait or []) > 1
            ):
                waits = list(si.on_wait)
                si.on_wait.clear()
                si.on_wait.append(waits[-1])
                for k, w in enumerate(waits[:-1]):
                    nop = mybir.InstNoOp(
                        name=f"{ins.name}-w{k}",
                        engine=ins.engine,
                        bass_nofuse=True,
                        sync_info=mybir.SyncInfo(on_wait=[w], on_update=[]),
                    )
                    insts.insert(i, nop)
                    i += 1
            i += 1


def _strip_redundant_dma_waits(nc):
    """walrus DMA structs accept a single sem wait, but Tile's sem pass emits
    two on pool-slot-recycling DMAs: (PE >= k) for the engine that consumed the
    slot's previous tile, plus (DMASWx >= v) for the WAW hazard vs the DMA that
    wrote that previous tile.  The PE wait strictly implies the DMA wait here
    (the consuming matmuls themselves waited on that DMA), so drop the DMA-sem
    wait.  Narrow on purpose: exactly-2 waits, one PE_*, one DMASW/DMAHW."""
    for blk in nc.m.functions[0].blocks:
        for ins in blk.instructions:
            if type(ins).__name__ != "InstDMACopy":
                continue
            si = getattr(ins, "sync_info", None)
            if si is None or not si.on_wait or len(si.on_wait) != 2:
                continue
            eng = [w for w in si.on_wait if w.ant_name.startswith("PE_")]
            dma = [w for w in si.on_wait if w.ant_name.startswith(("DMASW", "DMAHW"))]
            if len(eng) == 1 and len(dma) == 1:
                si.on_wait.remove(dma[0])
    for blk in nc.m.functions[0].blocks:
        for ins in blk.instructions:
            if type(ins).__name__ == "InstDMACopy":
                si = getattr(ins, "sync_info", None)
                assert si is None or len(si.on_wait or []) <= 1, ins.name


def _hi_lo(v64):
    hi = v64.astype(BF16)
    lo = (v64 - hi.astype(np.float64)).astype(BF16)
    return hi, lo


def _prep_shards(C, D):
    Cf = np.ascontiguousarray(np.asarray(C, dtype=np.float32).reshape(N, DDIM))
    Df = np.ascontiguousarray(np.asarray(D, dtype=np.float32).reshape(N, DDIM))

    c_sq = np.einsum("nd,nd->n", Cf, Cf, dtype=np.float64)
    d_sq = np.einsum("nd,nd->n", Df, Df, dtype=np.float64)

    A = np.zeros((DAUG, N), dtype=BF16)
    A[:DDIM] = Cf.astype(BF16).T
    dch, dcl = _hi_lo(c_sq)
    A[DDIM + 0] = dch
    A[DDIM + 1] = dcl
    A[DDIM + 2] = BF16(1)
    A[DDIM + 3] = BF16(1)

    B = np.zeros((DAUG, N), dtype=BF16)
    B[:DDIM] = (-2.0 * Df).astype(BF16).T
    ddh, ddl = _hi_lo(d_sq)
    B[DDIM + 0] = BF16(1)
    B[DDIM + 1] = BF16(1)
    B[DDIM + 2] = ddh
    B[DDIM + 3] = ddl

    # [DAUG, N] -> [G, P, CH, N]: group-major, partition-major inside a group,
    # so each per-group DMA is one fully contiguous read.
    A4 = np.ascontiguousarray(A.reshape(G, CH, P, N).transpose(0, 2, 1, 3))
    B4 = np.ascontiguousarray(B.reshape(G, CH, P, N).transpose(0, 2, 1, 3))

    ct_shards = [np.ascontiguousarray(A4[..., i * NI:(i + 1) * NI]) for i in range(4)]
    dt_shards = [np.ascontiguousarray(B4[..., j * NJ:(j + 1) * NJ]) for j in range(2)]
    return ct_shards, dt_shards


_NC_CACHE = {}


def _get_nc():
    if "nc" not in _NC_CACHE:
        _NC_CACHE["nc"] = _build_nc()
    return _NC_CACHE["nc"]


def _run(C, D, trace=False):
    from concourse.bass_utils import run_bass_kernel_spmd

    ct_shards, dt_shards = _prep_shards(C, D)
    in_maps = [
        {"ct": ct_shards[c // 2], "dt": dt_shards[c % 2]} for c in range(NCORES)
    ]
    res = run_bass_kernel_spmd(
        _get_nc(), in_maps, list(range(NCORES)), trace=trace
    )
    total = np.float64(0.0)
    for r in res.results:
        total += r["out"].astype(np.float64).sum()
    mean = total / (float(N) * float(N))
    return np.float32(mean), res


def kernel(C, D):
    val, _ = _run(C, D, trace=False)
    return np.asarray(val, dtype=np.float32)


# revision 11
# speedup vs baseline: 1.6020x; 1.6020x over previous
"""Euclidean distance loss (mean over all pairs ||C[i]-D[j]||_F) on 8 TRN2 cores.

Strategy:
  mean_ij ||C_i - D_j|| with ||c-d||^2 = ||c||^2 + ||d||^2 - 2<c,d>.
  The gram term is one big GEMM: [1024 x 16384] @ [16384 x 1024].

  Augmented-GEMM trick: the exact row norms (fp64, split hi/lo into bf16)
  ride along as 4 extra contraction rows in a tiny bf16 matmul that
  accumulates into the same PSUM tile as the gram, so PSUM directly holds
  ||c||^2 + ||d||^2 - 2<c,d> and the epilogue is a single scalar-engine
  sqrt-activation with free-dim accumulation per PSUM tile.

  The gram itself runs in fp8e4m3 with perf_mode=DoubleRow (two K-rows per
  PE cell): 64 K-chunks of 256, one matmul per chunk per i-subblock.  fp8
  halves HBM traffic vs bf16 (~12.8 MB/core) and DoubleRow halves PE
  streaming time.  Error analysis: the norms are exact, and fp8 quantization
  noise on the <c,d> term is zero-mean (quantization error of c is
  independent of d), so the mean over 2^20 pairs keeps ~1e-6 relative error.

  Sharding: 4 i-blocks (256 rows of C) x 2 j-blocks (512 rows of D) over the
  8 cores; host pre-transposes to [d, n] layout so both operands land with
  the contraction dim on partitions via fully contiguous HWDGE DMAs, with
  ramped group sizes ([1,1,2,4,8...]) so the PE starts within ~1.5 us.
"""

import sys
import numpy as np

for _p in ("/opt/trn_rl_repo", "/root/.axon_site/_ro/trn_rl_repo"):
    if _p not in sys.path:
        sys.path.insert(0, _p)

import ml_dtypes

BF16 = ml_dtypes.bfloat16
FP8 = ml_dtypes.float8_e4m3

N = 1024            # rows of C and of D
DDIM = 128 * 128    # flattened feature dim = 16384
P = 128             # SBUF partitions
KC = 256            # contraction rows per DoubleRow chunk (2 per partition)
NCHUNKS = DDIM // KC            # 64
GROUP_SIZES = [1, 1, 2, 4, 8, 8, 8, 8, 8, 8, 8]   # chunks per DMA group
assert sum(GROUP_SIZES) == NCHUNKS
NAUG = 4            # bf16 augmentation rows carrying the exact norms
NI = 256            # i-columns per core (4 i-blocks)
NJ = 512            # j-columns per core (2 j-blocks)
NCORES = 8


def _build_nc(hw=True):
    import concourse.bass as bass
    import concourse.mybir as mybir
    import concourse.tile as tile

    fp8 = mybir.dt.float8e4
    bf16 = mybir.dt.bfloat16
    f32 = mybir.dt.float32
    dr = mybir.MatmulPerfMode.DoubleRow

    nc = bass.Bass("TRN2")
    ct_ds = [
        nc.dram_tensor(f"ct{g}", [P, gs, 2, NI], fp8, kind="ExternalInput")
        for g, gs in enumerate(GROUP_SIZES)
    ]
    dt_ds = [
        nc.dram_tensor(f"dt{g}", [P, gs, 2, NJ], fp8, kind="ExternalInput")
        for g, gs in enumerate(GROUP_SIZES)
    ]
    cta_d = nc.dram_tensor("cta", [NAUG, NI], bf16, kind="ExternalInput")
    dta_d = nc.dram_tensor("dta", [NAUG, NJ], bf16, kind="ExternalInput")
    out_d = nc.dram_tensor("out", [P, 2], f32, kind="ExternalOutput")

    with tile.TileContext(nc) as tc:
        with (
            tc.tile_pool(name="ctp", bufs=4) as ct_pool,
            tc.tile_pool(name="dtp", bufs=4) as dt_pool,
            tc.tile_pool(name="aug", bufs=1) as aug_pool,
            tc.tile_pool(name="eps", bufs=1) as eps_pool,
            tc.tile_pool(name="ps", bufs=1, space=bass.MemorySpace.PSUM) as psum_pool,
        ):
            ps0 = psum_pool.tile([P, NJ], f32)
            ps1 = psum_pool.tile([P, NJ], f32)
            cta_t = aug_pool.tile([NAUG, NI], bf16)
            dta_t = aug_pool.tile([NAUG, NJ], bf16)
            nc.sync.dma_start(cta_t[:], cta_d[:])
            nc.sync.dma_start(dta_t[:], dta_d[:])
            k = 0
            for g, gs in enumerate(GROUP_SIZES):
                ct_t = ct_pool.tile([P, 8, 2, NI], fp8, tag="ct")
                dt_t = dt_pool.tile([P, 8, 2, NJ], fp8, tag="dt")
                nc.sync.dma_start(ct_t[:, :gs], ct_ds[g][:])
                nc.sync.dma_start(dt_t[:, :gs], dt_ds[g][:])
                for c in range(gs):
                    start = k == 0
                    nc.tensor.matmul(
                        ps0[:], ct_t[:, c, :, 0:128], dt_t[:, c, :, :],
                        start=start, stop=False, perf_mode=dr,
                    )
                    nc.tensor.matmul(
                        ps1[:], ct_t[:, c, :, 128:256], dt_t[:, c, :, :],
                        start=start, stop=False, perf_mode=dr,
                    )
                    k += 1
            # exact-norm augmentation rows (bf16) close both accumulations
            nc.tensor.matmul(
                ps0[:], cta_t[:, 0:128], dta_t[:], start=False, stop=True
            )
            nc.tensor.matmul(
                ps1[:], cta_t[:, 128:256], dta_t[:], start=False, stop=True
            )

            acc = eps_pool.tile([P, 2], f32)
            dist0 = eps_pool.tile([P, NJ], f32)
            dist1 = eps_pool.tile([P, NJ], f32)
            # bias=0.0 uses the preamble-initialized const AP (behind an
            # all-engine barrier), so the activations only wait on PE.
            nc.scalar.activation(
                dist0[:], ps0[:], mybir.ActivationFunctionType.Sqrt,
                bias=0.0, accum_out=acc[:, 0:1],
            )
            nc.scalar.activation(
                dist1[:], ps1[:], mybir.ActivationFunctionType.Sqrt,
                bias=0.0, accum_out=acc[:, 1:2],
            )
            nc.sync.dma_start(out_d[:], acc[:])

    if hw:
        # Post-passes that satisfy walrus' per-instruction sync-wait caps.
        # They only touch scheduling metadata (sem waits), not dataflow, but
        # the CoreSim race detector can't digest hand-inserted instructions,
        # so sim_test builds with hw=False.
        _strip_redundant_dma_waits(nc)
        _split_multiwait_drains(nc, mybir)
    return nc


def _strip_redundant_dma_waits(nc):
    """walrus DMA structs accept a single sem wait, but Tile's sem pass emits
    two on pool-slot-recycling DMAs: (PE >= k) for the engine that consumed the
    slot's previous tile, plus (DMAxx >= v) for the WAW hazard vs the DMA that
    wrote that previous tile.  The PE wait strictly implies the DMA wait here
    (the consuming matmuls themselves waited on that DMA), so drop the DMA-sem
    wait.  Narrow on purpose: exactly-2 waits, one PE_*, one DMASW/DMAHW."""
    for blk in nc.m.functions[0].blocks:
        for ins in blk.instructions:
            if type(ins).__name__ != "InstDMACopy":
                continue
            si = getattr(ins, "sync_info", None)
            if si is None or not si.on_wait or len(si.on_wait) != 2:
                continue
            eng = [
                w for w in si.on_wait
                if w.ant_name.startswith(("PE_", "Activation_"))
            ]
            dma = [w for w in si.on_wait if w.ant_name.startswith(("DMASW", "DMAHW"))]
            if len(eng) == 1 and len(dma) == 1:
                si.on_wait.remove(dma[0])
    for blk in nc.m.functions[0].blocks:
        for ins in blk.instructions:
            if type(ins).__name__ == "InstDMACopy":
                si = getattr(ins, "sync_info", None)
                assert si is None or len(si.on_wait or []) <= 1, ins.name


def _split_multiwait_drains(nc, mybir):
    """walrus CTRL structs also cap sync-wait commands per instruction; the
    Tile kernel-tail drain waits on every sem used.  Hoist all but the last
    wait onto single-wait NoOps queued immediately before the drain on the
    same engine — sequencer program order makes this equivalent."""
    for blk in nc.m.functions[0].blocks:
        insts = blk.instructions
        i = 0
        while i < len(insts):
            ins = insts[i]
            si = getattr(ins, "sync_info", None)
            if (
                type(ins).__name__ == "InstDrain"
                and si is not None
                and len(si.on_wait or []) > 1
            ):
                waits = list(si.on_wait)
                si.on_wait.clear()
                si.on_wait.append(waits[-1])
                for j, w in enumerate(waits[:-1]):
                    nop = mybir.InstNoOp(
                        name=f"{ins.name}-w{j}",
                        engine=ins.engine,
                        bass_nofuse=True,
                        sync_info=mybir.SyncInfo(on_wait=[w], on_update=[]),
                    )
                    insts.insert(i, nop)
                    i += 1
            i += 1


def _hi_lo(v64):
    hi = v64.astype(BF16)
    lo = (v64 - hi.astype(np.float64)).astype(BF16)
    return hi, lo


def _prep_shards(C, D):
    Cf = np.ascontiguousarray(np.asarray(C, dtype=np.float32).reshape(N, DDIM))
    Df = np.ascontiguousarray(np.asarray(D, dtype=np.float32).reshape(N, DDIM))

    c_sq = np.einsum("nd,nd->n", Cf, Cf, dtype=np.float64)
    d_sq = np.einsum("nd,nd->n", Df, Df, dtype=np.float64)

    # main gram rows, fp8, transposed to [d, n]
    A = np.ascontiguousarray(Cf.astype(FP8).T)           # [DDIM, N]
    B = np.ascontiguousarray((-2.0 * Df).astype(FP8).T)  # [DDIM, N]

    # DoubleRow layout: chunk c, partition p, slot i, col n <- row c*256+i*128+p
    # [DDIM, N] -> [NCHUNKS, 2, P, N] -> [NCHUNKS, P, 2, N]
    A4 = np.ascontiguousarray(A.reshape(NCHUNKS, 2, P, N).transpose(0, 2, 1, 3))
    B4 = np.ascontiguousarray(B.reshape(NCHUNKS, 2, P, N).transpose(0, 2, 1, 3))

    dch, dcl = _hi_lo(c_sq)
    ddh, ddl = _hi_lo(d_sq)
    Aaug = np.zeros((NAUG, N), dtype=BF16)
    Aaug[0], Aaug[1], Aaug[2], Aaug[3] = dch, dcl, BF16(1), BF16(1)
    Baug = np.zeros((NAUG, N), dtype=BF16)
    Baug[0], Baug[1], Baug[2], Baug[3] = BF16(1), BF16(1), ddh, ddl

    # per-group, group-local partition-major so every DMA is one
    # contiguous read: group g covers chunks [off, off+gs) -> [P, gs, 2, cols]
    def group_shards(M4, nsh, width):
        shards = []
        for s in range(nsh):
            cols = slice(s * width, (s + 1) * width)
            per_group = []
            off = 0
            for gs in GROUP_SIZES:
                blk = M4[off:off + gs, :, :, cols]          # [gs, P, 2, w]
                per_group.append(
                    np.ascontiguousarray(blk.transpose(1, 0, 2, 3))  # [P, gs, 2, w]
                )
                off += gs
            shards.append(per_group)
        return shards

    ct_shards = group_shards(A4, 4, NI)   # [4 shards][11 groups]
    dt_shards = group_shards(B4, 2, NJ)
    cta = [np.ascontiguousarray(Aaug[:, s * NI:(s + 1) * NI]) for s in range(4)]
    dta = [np.ascontiguousarray(Baug[:, s * NJ:(s + 1) * NJ]) for s in range(2)]
    return ct_shards, dt_shards, cta, dta


_NC_CACHE = {}


def _get_nc():
    if "nc" not in _NC_CACHE:
        _NC_CACHE["nc"] = _build_nc()
    return _NC_CACHE["nc"]


def _run(C, D, trace=False):
    from concourse.bass_utils import run_bass_kernel_spmd

    ct_shards, dt_shards, cta, dta = _prep_shards(C, D)
    in_maps = []
    for c in range(NCORES):
        pi, qi = c // 2, c % 2
        m = {"cta": cta[pi], "dta": dta[qi]}
        for g in range(len(GROUP_SIZES)):
            m[f"ct{g}"] = ct_shards[pi][g]
            m[f"dt{g}"] = dt_shards[qi][g]
        in_maps.append(m)
    res = run_bass_kernel_spmd(
        _get_nc(), in_maps, list(range(NCORES)), trace=trace
    )
    total = np.float64(0.0)
    for r in res.results:
        total += r["out"].astype(np.float64).sum()
    mean = total / (float(N) * float(N))
    return np.float32(mean), res


def kernel(C, D):
    val, _ = _run(C, D, trace=False)
    return np.asarray(val, dtype=np.float32)
